# revision 8
# baseline (speedup 1.0000x reference)
"""Trainium2 Bass kernel for nn_LinearReg_55508157333593.

Computes: loss = (c_omega * 0.001 / N) * sum over all rows/groups of
L2 norms of 25-element groups of weight [100000, 800] f32.

The whole buffer is 3.2M consecutive 25-float groups; we shard the flat
array across 8 NeuronCores (10M floats each) and stream each core's slab
through SBUF as [128, 78125] (each partition owns 3125 consecutive groups).

Raw-Bass manual pipeline, one DVE chain per chunk i:
  SP:  DMA chunk i into ring slot i%B at column base+1 (base col = 0 anchor)
  DVE: custom scan op (running sum of squares, in place), then a strided
       subtract: gs[g] = r[25(g+1)] - r[25g]  (anchor col 0 makes g=0 work)
  ACT: per chunk, sqrt over that chunk's gs slice with a fused per-partition
       row-sum (accum_out -> pr column); runs in ACT's idle time.
Endgame: after the last (tiny) chunk's sqrt, SP DMAs pr [128, n_chunks]
out; the host sums everything in float64 and applies the scaling.

The chunk schedule descends toward the end so the DVE chain drains in
lockstep with the DMA stream (DVE work/float ~0.96ns vs DMA ~1.20ns).
"""

import sys

import numpy as np

if "/opt/trn_rl_repo" not in sys.path:
    sys.path.insert(0, "/opt/trn_rl_repo")

N_CORES = 8
P = 128                      # SBUF partitions
GROUP = 25                   # elements per group
C_OMEGA = 0.001
N_ROWS = 100000
ROW = 800                    # elements per row
F_PER_PART = (N_ROWS * ROW) // (N_CORES * P)   # 78125 floats/partition/core

# chunk schedule (floats per partition; multiples of GROUP, sums to 78125):
# bulk streaming chunks, then a linear descent sized so the DVE scan chain
# drains with the stream, then one tiny chunk to shorten the final serial
# chain (last dma -> scan+diff -> sqrt -> out dma).
SCHEDULE = ([3125] * 15 + [2375] +
            [2400, 2300, 2200, 2125, 2025, 1925, 1825, 1750, 1650, 1550,
             1450, 1375, 1275, 1175, 1075, 1000, 900, 800] + [75])
IN_BUFS = 12

_compiled = None
LAST_RESULTS = None          # BassKernelResults of the most recent run

_SCAN_OP = None


def _get_scan_op():
    """Register (once) a custom DVE op: out = running sum of squares."""
    global _SCAN_OP
    if _SCAN_OP is not None:
        return _SCAN_OP
    from concourse import dve_ops
    from concourse.dve_spec import AluOp, Spec, Src0, lower, scan, sq
    from concourse.dve_uop import DveOpSpec

    name = "SUMSQ_SCAN_LREG"
    for op in dve_ops.OPS:
        if op.name == name:
            _SCAN_OP = op
            return op

    def _ref(in0, in1, s0, s1, imm2):
        return np.cumsum(
            np.asarray(in0, dtype=np.float32) ** 2, axis=-1, dtype=np.float32
        )

    spec = Spec(body=scan(AluOp.ADD, sq(Src0)), reference=_ref)
    row = dve_ops._CUSTOM_DVE_ROW_BASE + len(dve_ops.OPS)
    shas = {}
    for ver in ("v3", "v4"):
        try:
            shas[ver] = DveOpSpec(
                name=name, opcode=row, uops=lower(spec, ver=ver), rd1_en=False
            ).sha(ver)
        except Exception:
            pass
    op = dve_ops.DveOp(name, spec, subdim=False, uops_sha=shas)
    dve_ops.OPS.append(op)
    dve_ops.CUSTOM_DVE_SPECS[name] = spec
    dve_ops._SUB_OPCODE_FOR_NAME[name] = row
    _SCAN_OP = op
    return op


def build(f_per_part=F_PER_PART, schedule=None, in_bufs=IN_BUFS):
    """Build and compile the per-core raw-Bass program."""
    from concourse import bacc, mybir

    scan_op = _get_scan_op()

    if schedule is None:
        schedule = SCHEDULE
    n = len(schedule)
    assert sum(schedule) == f_per_part
    assert all(s % GROUP == 0 for s in schedule)
    offs = [sum(schedule[:i]) for i in range(n)]
    gpcs = [s // GROUP for s in schedule]
    goffs = [sum(gpcs[:i]) for i in range(n + 1)]
    total_g = goffs[n]
    max_sz = max(schedule)
    # col 0 of each slot = permanent 0.0 anchor; + GROUP-1 pad cols so the
    # strided `hi` view's nominal [g, 25] span stays inside the slot
    slot = max_sz + GROUP
    f32 = mybir.dt.float32
    Act = mybir.ActivationFunctionType
    B = in_bufs

    nc = bacc.Bacc("TRN2", target_bir_lowering=False, debug=False,
                   num_devices=N_CORES)
    x = nc.dram_tensor("x", [P, f_per_part], f32, kind="ExternalInput").ap()
    out = nc.dram_tensor("out", [P, n], f32, kind="ExternalOutput").ap()

    ring = nc.alloc_sbuf_tensor("ring", [P, B * slot], f32).ap()
    gs_all = nc.alloc_sbuf_tensor("gs_all", [P, total_g], f32).ap()
    pr = nc.alloc_sbuf_tensor("pr", [P, n], f32).ap()
    dm = nc.alloc_sbuf_tensor("dm_scratch", [1, 1], f32).ap()

    dma_sems = [nc.alloc_semaphore(f"dma_sem{b}") for b in range(B)]
    scan_sem = nc.alloc_semaphore("scan_sem")   # DVE memset/scan writes done
    red_sem = nc.alloc_semaphore("red_sem")     # DVE scan+diff for chunk done
    sqrt_sem = nc.alloc_semaphore("sqrt_sem")   # ACT sqrt piece done
    out_sem = nc.alloc_semaphore("out_sem")

    def base(c):
        return (c % B) * slot

    def emit_sp(sp):
        for i in range(n):
            if i >= B:
                sp.wait_ge(red_sem, i - B + 1)
            sp.dma_start(
                ring[:, base(i) + 1:base(i) + 1 + schedule[i]],
                x[:, offs[i]:offs[i] + schedule[i]],
            ).then_inc(dma_sems[i % B], 16)
        sp.wait_ge(sqrt_sem, n)
        sp.dma_start(out, pr).then_inc(out_sem, 16)
        sp.wait_ge(out_sem, 16)

    def emit_dve(dve):
        # zero the anchor column of every slot (before any diff reads it).
        # scan_sem edges are same-engine (HW orders DVE ops via the per-op
        # pipeline drain); they exist for the CoreSim race detector.
        dve.memset(ring.rearrange("p (b s) -> p b s", s=slot)[:, :, 0:1],
                   0.0).then_inc(scan_sem, 1)
        for c in range(n):
            sz, g = schedule[c], gpcs[c]
            data = ring[:, base(c) + 1:base(c) + 1 + sz]
            dve.wait_ge(dma_sems[c % B], 16 * (c // B + 1))
            dve._custom_dve(scan_op, out=data, in0=data).then_inc(scan_sem, 1)
            # group sums: r[25(g+1)] - r[25g]; col base(c) is the 0 anchor
            span = ring[:, base(c):base(c) + GROUP * g].rearrange(
                "p (g k) -> p g k", k=GROUP)
            hi = ring[:, base(c) + GROUP:base(c) + GROUP * (g + 1)].rearrange(
                "p (g k) -> p g k", k=GROUP)
            dve.wait_ge(scan_sem, c + 2)
            dve.tensor_tensor(
                gs_all[:, goffs[c]:goffs[c] + g],
                hi[:, :, 0:1],
                span[:, :, 0:1],
                mybir.AluOpType.subtract,
            ).then_inc(red_sem, 1)

    def emit_act(act):
        # table prefetch: a dummy Sqrt loads the activation table set early
        zero = nc.const_aps.aps[(f32, 0.0)]   # preamble-initialized [128, 1]
        act.activation(dm, zero[0:1, :], Act.Sqrt)
        for c in range(n):
            act.wait_ge(red_sem, c + 1)
            act.activation(
                gs_all[:, goffs[c]:goffs[c] + gpcs[c]],
                gs_all[:, goffs[c]:goffs[c] + gpcs[c]],
                Act.Sqrt,
                accum_out=pr[:, c:c + 1],
            ).then_inc(sqrt_sem, 1)

    emit_sp(nc.sync)
    emit_dve(nc.vector)
    emit_act(nc.scalar)

    nc.compile()
    return nc


def kernel(weight, c_omega):
    global _compiled, LAST_RESULTS
    from concourse.bass_utils import run_bass_kernel_spmd

    if _compiled is None:
        _compiled = build()
    nc = _compiled

    w = np.asarray(weight)
    if w.dtype != np.float32:
        w = w.astype(np.float32)
    w = np.ascontiguousarray(w)
    flat = w.reshape(-1)
    per_core = flat.size // N_CORES
    in_maps = [
        {"x": flat[c * per_core:(c + 1) * per_core].reshape(P, F_PER_PART)}
        for c in range(N_CORES)
    ]
    LAST_RESULTS = run_bass_kernel_spmd(nc, in_maps,
                                        core_ids=list(range(N_CORES)))
    total = 0.0
    for r in LAST_RESULTS.results:
        total += float(r["out"].astype(np.float64).sum())
    loss = total / N_ROWS * (C_OMEGA * float(c_omega))
    return np.float32(loss)


def selftest_sim(f_per_part=625, schedule=(250, 200, 125, 50), in_bufs=3,
                 seed=0):
    """CoreSim check on a scaled-down instance; returns max rel err."""
    from concourse.bass_interp import CoreSim

    nc = build(f_per_part=f_per_part, schedule=list(schedule),
               in_bufs=in_bufs)
    rng = np.random.default_rng(seed)
    xv = rng.standard_normal((P, f_per_part)).astype(np.float32)
    sim = CoreSim(nc)
    sim.tensor("x")[:] = xv
    sim.simulate()
    got = float(np.array(sim.tensor("out")).astype(np.float64).sum())
    g = xv.reshape(P, f_per_part // GROUP, GROUP)
    want = float(np.sqrt((g.astype(np.float64) ** 2).sum(-1)).sum())
    return abs(got - want) / abs(want)


# revision 10
# speedup vs baseline: 1.2142x; 1.2142x over previous
"""Trainium2 Bass kernel for nn_LinearReg_55508157333593.

Computes: loss = (c_omega * 0.001 / N) * sum over all rows/groups of
L2 norms of 25-element groups of weight [100000, 800] f32.

The whole buffer is 3.2M consecutive 25-float groups; we shard the flat
array across 8 NeuronCores (10M floats each) and stream each core's slab
through SBUF as [128, 78125] (each partition owns 3125 consecutive groups).

Raw-Bass manual pipeline, per chunk i:
  SP:  DMA chunk i into f32 ring slot i%B        (per-slot completion sems)
  ACT: square chunk i  f32 ring -> bf16 ring     (same slot index)
  DVE: per-group (25) reduce of the bf16 squares into gs_all [128, 3125] f32
ACT additionally runs sqrt pieces over finished spans of gs_all (fused
per-partition row-sum via accum_out -> pr column); these fill ACT's idle
time and only the last (tiny) piece is on the critical path. SP then DMAs
pr [128, n_pieces] out; the host sums everything in float64 and scales.

bf16 squares double DVE's reduce throughput (16-bit = 2 elem/cycle) so
DVE tracks the ~425 GB/s stream with slack; precision is far inside the
2e-2 gate. The chunk schedule descends at the end so both engines drain
in lockstep with the stream, and the final chunk/piece are tiny to keep
the post-stream serial chain (square -> reduce -> sqrt -> out DMA) short.

Equal slot counts in both rings make ACT's bf16-slot reuse ordering come
free: dma(i) already waited on reduce(i-B) via SP's f32-slot guard.
"""

import sys

import numpy as np

if "/opt/trn_rl_repo" not in sys.path:
    sys.path.insert(0, "/opt/trn_rl_repo")

N_CORES = 8
P = 128                      # SBUF partitions
GROUP = 25                   # elements per group
C_OMEGA = 0.001
N_ROWS = 100000
ROW = 800                    # elements per row
F_PER_PART = (N_ROWS * ROW) // (N_CORES * P)   # 78125 floats/partition/core

# chunk schedule (floats per partition; multiples of GROUP, sums to 78125)
SCHEDULE = ([3125] * 20 + [1550] +
            [2700, 2425, 2150, 1875, 1625, 1350, 1075, 800] + [75])
# sqrt piece boundaries (chunk indices; last == len(SCHEDULE)) and the chunk
# after whose square each piece is emitted in ACT program order
SEG_BOUNDS = [7, 14, 21, 26, 29, 30]
IN_BUFS = 10

_compiled = None
LAST_RESULTS = None          # BassKernelResults of the most recent run


def build(f_per_part=F_PER_PART, schedule=None, in_bufs=IN_BUFS,
          seg_bounds=None):
    """Build and compile the per-core raw-Bass program."""
    from concourse import bacc, mybir

    if schedule is None:
        schedule = SCHEDULE
        seg_bounds = SEG_BOUNDS
    n = len(schedule)
    if seg_bounds is None:
        seg_bounds = [max(1, n - 1), n] if n > 1 else [n]
    assert sum(schedule) == f_per_part
    assert all(s % GROUP == 0 for s in schedule)
    assert seg_bounds[-1] == n and sorted(seg_bounds) == seg_bounds
    offs = [sum(schedule[:i]) for i in range(n)]
    gpcs = [s // GROUP for s in schedule]
    goffs = [sum(gpcs[:i]) for i in range(n + 1)]
    total_g = goffs[n]
    max_sz = max(schedule)
    n_segs = len(seg_bounds)
    # (end_chunk, place_after_chunk, gstart, gend) per sqrt piece
    segs = []
    prev = 0
    for i, b in enumerate(seg_bounds):
        place = min(b + 1, n - 1) if i < n_segs - 1 else n - 1
        segs.append((b, place, goffs[prev], goffs[b]))
        prev = b
    f32 = mybir.dt.float32
    bf16 = mybir.dt.bfloat16
    Act = mybir.ActivationFunctionType
    B = in_bufs

    nc = bacc.Bacc("TRN2", target_bir_lowering=False, debug=False,
                   num_devices=N_CORES)
    x = nc.dram_tensor("x", [P, f_per_part], f32, kind="ExternalInput").ap()
    out = nc.dram_tensor("out", [P, n_segs], f32, kind="ExternalOutput").ap()

    ring = nc.alloc_sbuf_tensor("ring", [P, B * max_sz], f32).ap()
    sqr = nc.alloc_sbuf_tensor("sqr", [P, B * max_sz], bf16).ap()
    gs_all = nc.alloc_sbuf_tensor("gs_all", [P, total_g], f32).ap()
    pr = nc.alloc_sbuf_tensor("pr", [P, n_segs], f32).ap()
    dm = nc.alloc_sbuf_tensor("dm_scratch", [1, 1], f32).ap()

    dma_sems = [nc.alloc_semaphore(f"dma_sem{b}") for b in range(B)]
    sq_sem = nc.alloc_semaphore("sq_sem")       # ACT square i done
    red_sem = nc.alloc_semaphore("red_sem")     # DVE reduce i done
    sqrt_sem = nc.alloc_semaphore("sqrt_sem")   # ACT sqrt piece done
    out_sem = nc.alloc_semaphore("out_sem")

    def tile(buf, c):
        b = (c % B) * max_sz
        return buf[:, b:b + schedule[c]]

    def emit_sp(sp):
        for i in range(n):
            if i >= B:
                # f32 slot free once the reduce covering it completed
                sp.wait_ge(red_sem, i - B + 1)
            sp.dma_start(tile(ring, i),
                         x[:, offs[i]:offs[i] + schedule[i]]
                         ).then_inc(dma_sems[i % B], 16)
        sp.wait_ge(sqrt_sem, n_segs)
        sp.dma_start(out, pr).then_inc(out_sem, 16)
        sp.wait_ge(out_sem, 16)

    def emit_act(act):
        # table prefetch: first activation is a Sqrt so the one table set
        # loaded (sqrt_and_others, also contains Square) serves the kernel
        zero = nc.const_aps.aps[(f32, 0.0)]   # preamble-initialized [128, 1]
        act.activation(dm, zero[0:1, :], Act.Sqrt)

        place = {}
        for s_i, (b, pc, glo, ghi) in enumerate(segs):
            place.setdefault(pc, []).append((s_i, b, glo, ghi))
        for c in range(n):
            act.wait_ge(dma_sems[c % B], 16 * (c // B + 1))
            act.activation(tile(sqr, c), tile(ring, c),
                           Act.Square).then_inc(sq_sem, 1)
            for (s_i, b, glo, ghi) in place.get(c, []):
                act.wait_ge(red_sem, b)
                act.activation(gs_all[:, glo:ghi], gs_all[:, glo:ghi],
                               Act.Sqrt,
                               accum_out=pr[:, s_i:s_i + 1]
                               ).then_inc(sqrt_sem, 1)

    def emit_dve(dve):
        for c in range(n):
            dve.wait_ge(sq_sem, c + 1)
            dve.reduce_sum(
                gs_all[:, goffs[c]:goffs[c] + gpcs[c]],
                tile(sqr, c).rearrange("p (g k) -> p g k", k=GROUP),
                axis=mybir.AxisListType.X,
            ).then_inc(red_sem, 1)

    emit_sp(nc.sync)
    emit_act(nc.scalar)
    emit_dve(nc.vector)

    nc.compile()
    return nc


def kernel(weight, c_omega):
    global _compiled, LAST_RESULTS
    from concourse.bass_utils import run_bass_kernel_spmd

    if _compiled is None:
        _compiled = build()
    nc = _compiled

    w = np.asarray(weight)
    if w.dtype != np.float32:
        w = w.astype(np.float32)
    w = np.ascontiguousarray(w)
    flat = w.reshape(-1)
    per_core = flat.size // N_CORES
    in_maps = [
        {"x": flat[c * per_core:(c + 1) * per_core].reshape(P, F_PER_PART)}
        for c in range(N_CORES)
    ]
    LAST_RESULTS = run_bass_kernel_spmd(nc, in_maps,
                                        core_ids=list(range(N_CORES)))
    total = 0.0
    for r in LAST_RESULTS.results:
        total += float(r["out"].astype(np.float64).sum())
    loss = total / N_ROWS * (C_OMEGA * float(c_omega))
    return np.float32(loss)


def selftest_sim(f_per_part=625, schedule=(250, 200, 125, 50), in_bufs=3,
                 seg_bounds=(2, 4), seed=0):
    """CoreSim check on a scaled-down instance; returns rel err (bf16-limited)."""
    from concourse.bass_interp import CoreSim

    nc = build(f_per_part=f_per_part, schedule=list(schedule),
               in_bufs=in_bufs, seg_bounds=list(seg_bounds))
    rng = np.random.default_rng(seed)
    xv = rng.standard_normal((P, f_per_part)).astype(np.float32)
    sim = CoreSim(nc)
    sim.tensor("x")[:] = xv
    sim.simulate()
    got = float(np.array(sim.tensor("out")).astype(np.float64).sum())
    g = xv.reshape(P, f_per_part // GROUP, GROUP)
    want = float(np.sqrt((g.astype(np.float64) ** 2).sum(-1)).sum())
    return abs(got - want) / abs(want)
